# revision 22
# baseline (speedup 1.0000x reference)
"""Deformable conv (torchvision v1, stride=1 pad=1 K=3) on 8 TRN2 NeuronCores.

Sharding: core i handles sample b=i//2, output-row half i%2 (48 of 96 rows).
Weights replicated; no cross-core communication.

Per-core pipeline (v2, instruction-lean):
  A. offset conv (3x3, 256->18) via PE matmuls, PE-transposed to
     pixel-on-partition offs_sb [128, 36, 18] fp32.
  B. batched sampling math, all 9 taps per op ([128, 36, 9] fp32 tiles):
     floor via fp32 magic-round, bilinear corner weights with validity
     masks written straight into the broadcast-combine layout, clamped
     paired int16 gather indices, SWDGE-wrapped via strided DMAs.
  C. per (tap, 512-px block): dma_gather from DRAM xt2 (row q = the 512
     bf16 [x_T[q] | x_T[q+1]]) -> gt [128 px, 8, 512].
  D. combine: one broadcast tensor_tensor multiply + two adds ->
     val_T [128 px, 4 chunks, 2304 (k,c)] bf16.
  E. transpose to matmul layout via SBUF-source dma_gather
     (tokens_per_rank=128, identity indices) -> val_mm [128 c, 18 ct, 512 px].
  F. 18 accumulating bf16 matmuls per 512-px block -> psum [o, 512],
     + bias via ACT, DMA out.
"""

import sys

import numpy as np

if "/opt/trn_rl_repo" not in sys.path:
    sys.path.insert(0, "/opt/trn_rl_repo")

import ml_dtypes  # noqa: E402

bf16 = ml_dtypes.bfloat16

B, C, H, W, O = 4, 256, 96, 96, 256
XT_ROWS = 96 + 96 * 96 + 96
K, KK = 3, 9
HW = H * W
P = HW // 2                     # 4608 pixels per core
NCHUNK = P // 128               # 36
NBLK = P // 512                 # 9
ROWS = 48
CT = 2 * KK                     # 18 contraction tiles
MAGIC = 12582912.0              # 1.5 * 2^23


def _wrap_idx(idx):
    """logical list -> SWDGE wrapped [128, n/16] (replicated 16-groups)."""
    n = idx.shape[0]
    w = np.zeros((128, n // 16), np.int16)
    j = np.arange(n)
    w[j % 16, j // 16] = idx
    for g in range(1, 8):
        w[g * 16 : (g + 1) * 16] = w[0:16]
    return w


# ---------------------------------------------------------------- host prep
def _prep_core_inputs(x, offset_w, offset_b, deform_w, deform_b, core):
    b, half = core // 2, core % 2
    h0 = half * ROWS
    xb = x[b]                                       # [C, H, W] fp32

    xpad = np.zeros((2, 128, 50, 98), dtype=np.float32)
    r_lo, r_hi = h0 - 1, h0 + ROWS + 1
    src_lo, src_hi = max(r_lo, 0), min(r_hi, H)
    d_lo = src_lo - r_lo
    xpad[:, :, d_lo : d_lo + (src_hi - src_lo), 1:97] = xb[
        :, src_lo:src_hi, :
    ].reshape(2, 128, src_hi - src_lo, W)
    # xp3[kx]: columns shifted by kx-1, zero-padded; rows 50, cols 96
    xp = np.stack(
        [xpad[:, :, :, kx : kx + 96] for kx in range(3)], axis=0
    ).astype(bf16)

    # paired gather source: row r = [xTp[r] | xTp[r+1]], 96-row lead pad
    xtp = np.zeros((XT_ROWS + 1, 256), dtype=bf16)
    xtp[96 : 96 + HW] = xb.reshape(C, HW).T.astype(bf16)
    xt = np.concatenate([xtp[:XT_ROWS], xtp[1 : XT_ROWS + 1]], axis=1)

    ow = np.ascontiguousarray(
        offset_w.reshape(18, 2, 128, KK).transpose(1, 2, 3, 0).reshape(2, 128, KK * 18)
    ).astype(bf16)

    dw = np.ascontiguousarray(
        deform_w.reshape(O, 2, 128, KK).transpose(3, 1, 2, 0).reshape(CT, 128, O)
    ).astype(bf16)

    ob = np.ascontiguousarray(np.broadcast_to(offset_b.astype(np.float32), (128, 18)))
    db = np.ascontiguousarray(deform_b.reshape(2, 128, 1).astype(np.float32))

    p_local = np.arange(P)
    basey = (h0 + p_local // W).astype(np.float32).reshape(NCHUNK, 128).T
    basex = (p_local % W).astype(np.float32).reshape(NCHUNK, 128).T
    ky = (np.arange(KK) // 3 - 1).astype(np.float32)
    kx = (np.arange(KK) % 3 - 1).astype(np.float32)
    yb = np.ascontiguousarray(basey[:, :, None] + ky[None, None, :])  # [128,36,9]
    xb_all = np.ascontiguousarray(basex[:, :, None] + kx[None, None, :])

    identf = np.eye(128, dtype=np.float32)

    # identity permutation for the SBUF transpose-gather: pixel j -> idx j
    xwt = _wrap_idx(np.arange(512).astype(np.int16))

    return {
        "xp": np.ascontiguousarray(xp.reshape(3, 2, 128, 50 * 96)),
        "xt": xt,
        "ow": ow,
        "dw": dw,
        "ob": ob,
        "db": db,
        "yb": yb,
        "xb": xb_all,
        "identf": identf,
        "xwt": xwt,
    }


# ---------------------------------------------------------------- bass build
def build_nc(repeat=1):
    import concourse.bass as bass
    from concourse import bacc, tile

    mybir = bass.mybir
    dt = mybir.dt
    Alu = mybir.AluOpType
    Act = mybir.ActivationFunctionType

    nc = bacc.Bacc()

    xp_d = nc.declare_dram_parameter("xp", [3, 2, 128, 50 * 96], dt.bfloat16, isOutput=False)
    xt_d = nc.declare_dram_parameter("xt", [XT_ROWS, 512], dt.bfloat16, isOutput=False)
    ow_d = nc.declare_dram_parameter("ow", [2, 128, KK * 18], dt.bfloat16, isOutput=False)
    dw_d = nc.declare_dram_parameter("dw", [CT, 128, O], dt.bfloat16, isOutput=False)
    ob_d = nc.declare_dram_parameter("ob", [128, 18], dt.float32, isOutput=False)
    db_d = nc.declare_dram_parameter("db", [2, 128, 1], dt.float32, isOutput=False)
    yb_d = nc.declare_dram_parameter("yb", [128, NCHUNK, KK], dt.float32, isOutput=False)
    xb_d = nc.declare_dram_parameter("xb", [128, NCHUNK, KK], dt.float32, isOutput=False)
    identf_d = nc.declare_dram_parameter("identf", [128, 128], dt.float32, isOutput=False)
    xwt_d = nc.declare_dram_parameter("xwt", [128, 32], dt.int16, isOutput=False)
    out_d = nc.declare_dram_parameter("out", [2, 128, P], dt.float32, isOutput=True)

    reg1024 = nc.gpsimd.to_reg(1024)
    reg512 = nc.gpsimd.to_reg(512)

    with tile.TileContext(nc) as tc:
        from contextlib import ExitStack

        with ExitStack() as ctx:
            cst = ctx.enter_context(tc.tile_pool(name="cst", bufs=1))
            sb = ctx.enter_context(tc.tile_pool(name="sb", bufs=1))
            g_pool = ctx.enter_context(tc.tile_pool(name="gth", bufs=1))
            vt_pool = ctx.enter_context(tc.tile_pool(name="vt", bufs=1))
            val_pool = ctx.enter_context(tc.tile_pool(name="val", bufs=1))
            vm_pool = ctx.enter_context(tc.tile_pool(name="vm", bufs=1))
            outp = ctx.enter_context(tc.tile_pool(name="outp", bufs=2))
            ps_t = ctx.enter_context(tc.tile_pool(name="ps_t", bufs=2, space="PSUM"))
            ps_mm = ctx.enter_context(tc.tile_pool(name="ps_mm", bufs=4, space="PSUM"))
            ps_oc = ctx.enter_context(tc.tile_pool(name="ps_oc", bufs=2, space="PSUM"))

            # ---- constants / weights
            xp_sb = cst.tile([128, 3, 2, 50 * 96], dt.bfloat16)
            for kx in range(3):
                for t in range(2):
                    nc.sync.dma_start(out=xp_sb[:, kx, t], in_=xp_d[kx, t])
            ow_sb = cst.tile([128, 2, KK * 18], dt.bfloat16)
            for t in range(2):
                nc.sync.dma_start(out=ow_sb[:, t], in_=ow_d[t])
            dw_sb = cst.tile([128, CT, O], dt.bfloat16)
            for ct_i in range(CT):
                nc.sync.dma_start(out=dw_sb[:, ct_i], in_=dw_d[ct_i])
            ob_sb = cst.tile([128, 18], dt.float32)
            nc.sync.dma_start(out=ob_sb[:], in_=ob_d[:])
            db_sb = cst.tile([128, 2], dt.float32)
            for t in range(2):
                nc.sync.dma_start(out=db_sb[:, t : t + 1], in_=db_d[t])
            yb_sb = cst.tile([128, NCHUNK, KK], dt.float32)
            nc.sync.dma_start(out=yb_sb[:], in_=yb_d[:])
            xb_sb = cst.tile([128, NCHUNK, KK], dt.float32)
            nc.sync.dma_start(out=xb_sb[:], in_=xb_d[:])
            identf_sb = cst.tile([128, 128], dt.float32)
            nc.sync.dma_start(out=identf_sb[:], in_=identf_d[:])
            xwt_sb = cst.tile([128, 32], dt.int16)
            nc.sync.dma_start(out=xwt_sb[:], in_=xwt_d[:])

            for _rep in range(repeat):
                # ---- stage A: offset conv -> offs_sb [128, 36, 18] fp32
                offs_sb = sb.tile([128, NCHUNK, 18], dt.float32)
                for g in range(NBLK):                 # 512-px groups, N=512
                    ps = ps_oc.tile([18, 512], dt.float32, tag="psoc")
                    n = 0
                    for t in range(2):
                        for k in range(KK):
                            ky, kx = k // 3, k % 3
                            rhs = xp_sb[:, kx, t, g * 512 + ky * 96 : g * 512 + ky * 96 + 512]
                            nc.tensor.matmul(
                                ps[:],
                                lhsT=ow_sb[:, t, k * 18 : (k + 1) * 18],
                                rhs=rhs,
                                start=(n == 0),
                                stop=(n == 17),
                            )
                            n += 1
                    oc_sb = sb.tile([18, 512], dt.float32, tag="ocsb")
                    nc.scalar.copy(oc_sb[:], ps[:])
                    for t3 in range(4):
                        pst = ps_t.tile([128, 18], dt.float32, tag="pst")
                        nc.tensor.transpose(
                            pst[:],
                            oc_sb[:, t3 * 128 : (t3 + 1) * 128],
                            identf_sb[:18, :18],
                        )
                        nc.vector.tensor_tensor(
                            offs_sb[:, g * 4 + t3], pst[:], ob_sb[:], Alu.add
                        )

                # ---- stage B: batched sampling math over [128, 36, 9]
                scr = sb.tile([128, 10, NCHUNK, KK], dt.float32)
                w_exp = sb.tile([128, KK, NBLK, 2, 4, 2], dt.float32)
                qf_sb = sb.tile([128, 2, NCHUNK, KK], dt.float32)
                qi_sb = sb.tile([128, KK, NBLK, 2, 4], dt.int16)
                xw_sb = sb.tile([128, KK, NBLK, 64], dt.int16)

                py, px = scr[:, 0], scr[:, 1]
                ty, tx = scr[:, 2], scr[:, 3]
                y0, x0 = scr[:, 4], scr[:, 5]
                t1 = scr[:, 6]
                vy0, vy1 = scr[:, 7], scr[:, 8]
                wtmp = scr[:, 9]

                dy = offs_sb.rearrange("p n (k two) -> p n k two", two=2)[:, :, :, 0]
                dx = offs_sb.rearrange("p n (k two) -> p n k two", two=2)[:, :, :, 1]
                nc.vector.tensor_tensor(py, dy, yb_sb[:], Alu.add)
                nc.vector.tensor_tensor(px, dx, xb_sb[:], Alu.add)
                for (pp, tt, zz) in ((py, ty, y0), (px, tx, x0)):
                    nc.vector.tensor_scalar(
                        out=t1, in0=pp, scalar1=0.49999997, scalar2=MAGIC,
                        op0=Alu.subtract, op1=Alu.add,
                    )
                    nc.vector.tensor_scalar(
                        out=zz, in0=t1, scalar1=MAGIC, scalar2=None, op0=Alu.subtract
                    )
                    nc.vector.tensor_tensor(tt, pp, zz, Alu.subtract)

                # validity-weighted 1d weights -> corners into w_exp
                # w_exp[p, k, B, tb, ch, h] ; chunk = B*4+ch ; corner=(tb,h)
                def corner_out(tb, h):
                    return w_exp[:, :, :, tb, :, h]          # [p, k, nb, ch]

                def kview(t):
                    return t.rearrange("p (nb ch) k -> p k nb ch", ch=4)

                for tb, (lo, hi) in enumerate(((0.0, 95.0), (-1.0, 94.0))):
                    v = vy0 if tb == 0 else vy1
                    nc.vector.tensor_scalar(out=t1, in0=y0, scalar1=lo, scalar2=None, op0=Alu.is_ge)
                    nc.vector.tensor_scalar(out=v, in0=y0, scalar1=hi, scalar2=None, op0=Alu.is_le)
                    nc.vector.tensor_tensor(v, v, t1, Alu.mult)
                nc.vector.tensor_scalar(out=wtmp, in0=ty, scalar1=-1.0, scalar2=1.0, op0=Alu.mult, op1=Alu.add)
                nc.vector.tensor_tensor(vy0, vy0, wtmp, Alu.mult)   # (1-ty)*vy0
                nc.vector.tensor_tensor(vy1, vy1, ty, Alu.mult)     # ty*vy1
                wx0, wx1 = scr[:, 2], scr[:, 6]  # reuse ty/t1 slots
                nc.vector.tensor_scalar(out=wtmp, in0=x0, scalar1=0.0, scalar2=None, op0=Alu.is_ge)
                nc.vector.tensor_scalar(out=wx0, in0=x0, scalar1=95.0, scalar2=None, op0=Alu.is_le)
                nc.vector.tensor_tensor(wx0, wx0, wtmp, Alu.mult)
                nc.vector.tensor_scalar(out=wtmp, in0=tx, scalar1=-1.0, scalar2=1.0, op0=Alu.mult, op1=Alu.add)
                nc.vector.tensor_tensor(wx0, wx0, wtmp, Alu.mult)   # (1-tx)*vx0
                nc.vector.tensor_scalar(out=wtmp, in0=x0, scalar1=-1.0, scalar2=None, op0=Alu.is_ge)
                nc.vector.tensor_scalar(out=wx1, in0=x0, scalar1=94.0, scalar2=None, op0=Alu.is_le)
                nc.vector.tensor_tensor(wx1, wx1, wtmp, Alu.mult)
                nc.vector.tensor_tensor(wx1, wx1, tx, Alu.mult)     # tx*vx1

                nc.vector.tensor_tensor(corner_out(0, 0), kview(vy0), kview(wx0), Alu.mult)
                nc.vector.tensor_tensor(corner_out(0, 1), kview(vy0), kview(wx1), Alu.mult)
                nc.vector.tensor_tensor(corner_out(1, 0), kview(vy1), kview(wx0), Alu.mult)
                nc.vector.tensor_tensor(corner_out(1, 1), kview(vy1), kview(wx1), Alu.mult)

                # paired gather indices: xc in [-1, 95], +96 lead pad
                yc0, yc1, xc0 = scr[:, 3], scr[:, 7], scr[:, 8]  # reuse
                nc.vector.tensor_scalar(out=yc0, in0=y0, scalar1=0.0, scalar2=95.0, op0=Alu.max, op1=Alu.min)
                nc.vector.tensor_scalar(out=yc1, in0=y0, scalar1=1.0, scalar2=0.0, op0=Alu.add, op1=Alu.max)
                nc.vector.tensor_scalar(out=yc1, in0=yc1, scalar1=95.0, scalar2=None, op0=Alu.min)
                nc.vector.tensor_scalar(out=xc0, in0=x0, scalar1=-1.0, scalar2=95.0, op0=Alu.max, op1=Alu.min)
                nc.vector.tensor_scalar(out=xc0, in0=xc0, scalar1=96.0, scalar2=None, op0=Alu.add)
                nc.vector.scalar_tensor_tensor(out=qf_sb[:, 0], in0=yc0, scalar=96.0, in1=xc0, op0=Alu.mult, op1=Alu.add)
                nc.vector.scalar_tensor_tensor(out=qf_sb[:, 1], in0=yc1, scalar=96.0, in1=xc0, op0=Alu.mult, op1=Alu.add)
                # int16 + reorder (tb, (nb ch), k) -> (k, nb, tb, ch)
                nc.vector.tensor_copy(
                    qi_sb.rearrange("p k nb tb ch -> p tb nb ch k"),
                    qf_sb.rearrange("p tb (nb ch) k -> p tb nb ch k", nb=NBLK),
                )
                # SWDGE wrap: within (k,B): col c = tb*32 + ch*8 + g
                for g8 in range(8):
                    src = qi_sb[g8 * 16 : (g8 + 1) * 16].rearrange(
                        "p k nb tb ch -> p (k nb tb ch)"
                    )
                    dst = xw_sb[0:16].rearrange(
                        "p k nb (m g) -> p (k nb m) g", g=8
                    )[:, :, g8]
                    nc.sync.dma_start(out=dst, in_=src)
                for g8 in range(1, 8):
                    nc.sync.dma_start(
                        out=xw_sb[g8 * 16 : (g8 + 1) * 16], in_=xw_sb[0:16]
                    )

                # ---- stages C-F per 512-px block
                for Bb in range(NBLK):
                    val_sb = val_pool.tile([128, 4, CT * 128], dt.bfloat16, tag="val")
                    for kp in range(3):               # tap triples
                        k = 3 * kp
                        nk = 3
                        gt = g_pool.tile([128, 3, 8, 512], dt.bfloat16, tag="g")
                        for i in range(nk):
                            nc.gpsimd.dma_gather(
                                out_ap=gt[:, i],
                                in_ap=xt_d[:],
                                idxs_ap=xw_sb[:, k + i, Bb],
                                num_idxs=1024,
                                num_idxs_reg=reg1024,
                                elem_size=512,
                            )
                        # combine: m = gt * w (broadcast), fold tb then h
                        wb = (
                            w_exp[:, k : k + nk, Bb]
                            .unsqueeze(5)
                            .broadcast_to((128, nk, 2, 4, 2, 256))
                        )
                        m = vt_pool.tile([128, 3, 2, 4, 2, 256], dt.bfloat16, tag="m")
                        nc.vector.tensor_tensor(
                            m[:, :nk],
                            gt[:, :nk].rearrange(
                                "p kk (tb ch) (h c) -> p kk tb ch h c", tb=2, h=2
                            ),
                            wb,
                            Alu.mult,
                        )
                        s = vt_pool.tile([128, 3, 4, 2, 256], dt.bfloat16, tag="s")
                        nc.vector.tensor_tensor(
                            s[:, :nk], m[:, :nk, 0], m[:, :nk, 1], Alu.add
                        )
                        nc.vector.tensor_tensor(
                            val_sb[:, :, k * 256 : (k + nk) * 256].rearrange(
                                "p ch (kk c) -> p kk ch c", kk=nk
                            ),
                            s[:, :nk, :, 0],
                            s[:, :nk, :, 1],
                            Alu.add,
                        )
                    # transpose to matmul layout via SBUF-source gather
                    vm = vm_pool.tile([128, CT, 512], dt.bfloat16, tag="vm")
                    nc.gpsimd.dma_gather(
                        out_ap=vm[:],
                        in_ap=val_sb[:],
                        idxs_ap=xwt_sb[:],
                        num_idxs=512,
                        num_idxs_reg=reg512,
                        elem_size=CT * 128,
                        transpose=True,
                        sbuf_tokens_per_rank=128,
                        sbuf_free_dim_per_rank=CT * 128 * 2,
                    )
                    for oh in range(2):
                        pm = ps_mm.tile([128, 512], dt.float32, tag="pm")
                        for ct_i in range(CT):
                            nc.tensor.matmul(
                                pm[:],
                                lhsT=dw_sb[:, ct_i, oh * 128 : (oh + 1) * 128],
                                rhs=vm[:, ct_i],
                                start=(ct_i == 0),
                                stop=(ct_i == CT - 1),
                            )
                        ob_t = outp.tile([128, 512], dt.float32, tag="ot")
                        nc.scalar.activation(
                            out=ob_t[:], in_=pm[:],
                            func=Act.Identity, bias=db_sb[:, oh : oh + 1], scale=1.0,
                        )
                        nc.sync.dma_start(
                            out=out_d[oh, :, Bb * 512 : (Bb + 1) * 512], in_=ob_t[:]
                        )

    nc.compile()
    return nc


# ------------------------------------------------------------ main entry
_NC_CACHE = {}


def _get_nc():
    if "nc" not in _NC_CACHE:
        _NC_CACHE["nc"] = build_nc()
    return _NC_CACHE["nc"]


def _assemble(results):
    out = np.empty((B, O, H, W), dtype=np.float32)
    for core in range(8):
        b, half = core // 2, core % 2
        o = np.asarray(results[core]["out"]).reshape(O, ROWS, W)
        out[b, :, half * ROWS : (half + 1) * ROWS, :] = o
    return out


def kernel(x, offset_w, offset_b, deform_w, deform_b, **_ignored):
    from concourse.bass_utils import run_bass_kernel_spmd

    x = np.asarray(x, dtype=np.float32)
    offset_w = np.asarray(offset_w, dtype=np.float32)
    offset_b = np.asarray(offset_b, dtype=np.float32)
    deform_w = np.asarray(deform_w, dtype=np.float32)
    deform_b = np.asarray(deform_b, dtype=np.float32)

    nc = _get_nc()
    in_maps = [
        _prep_core_inputs(x, offset_w, offset_b, deform_w, deform_b, core)
        for core in range(8)
    ]
    res = run_bass_kernel_spmd(nc, in_maps, core_ids=list(range(8)))
    return _assemble([res.results[i] for i in range(8)])
